# revision 13
# baseline (speedup 1.0000x reference)
"""NCD-via-LZW kernel for Trainium2 (8 NeuronCores, Bass) — v3.

Problem: quantize x [32,3,32,32] to 8 levels along a space-filling curve =>
96 strings of length 1024; LZW-compress the 96 strings, the 48 pattern maps,
and the 1536 string||pmap concatenations; return the normalized compression
distance matrix [32, 48].

Mapping: LZW is sequential per sequence but there are 1680 independent
sequences. Each NeuronCore handles batches 4n..4n+3 (192 concat runs) plus 6
of the 48 pmap runs, one LZW sequence per SBUF partition, on the Vector
engine. Wave 0 runs the full 2047-step concat string||pmap_{k0}; wave 1
reuses the trie state wave 0 left in SBUF (slots [0,1024) are exactly the
state after the shared 1024-symbol string prefix) and runs only the 1024
suffix steps for pmap_{k1}, overwriting slots [1024,2048) just ahead of its
own match stream.

v3 vs v1: the per-entry node id is its insertion slot + 8 (static), so the
match scan multiplies by a precomputed IOTA array instead of a maintained
EN array — 3 instructions per step instead of 4; compressed sizes are
end-of-run nonzero counts of EK instead of EN snapshots.

Per-lane LZW state (all exact in fp32):
  key(cur, c) = cur + (c+1)/16  (cur = trie node id, c in [0,8))
  EK[t] = key inserted at step t on miss, 0.0 on hit (queries >= 1/16 > 0)
Step t (query in key, match result in acc):
  1. acc = sum_j (EK[0:t] == key) * IOTA[0:t]     (IOTA[j] = j+8)
  2. EK[t] = (acc == 0) * key
  3. key = max(acc, c_t) + (c_{t+1}+1)/16         (node ids >= 8 > c_t)
lzw_count = 1 + nnz(EK[0:L]).
"""

import numpy as np

B, C, H, W = 32, 3, 32, 32
L = 8
P = 16
M = 1024
N = H * W
T = 2048
PRE = 1024
NCORES = 8

_nc_cache = {}


class _Chain:
    """Same-engine serialization via an attached-wait semaphore chain
    (required for correctness: back-to-back RAW on SBUF races the write
    ack — verified empirically)."""

    def __init__(self, sem):
        self.sem = sem
        self.k = 0

    def add(self, inst, wait=True):
        """wait=False skips the wait (still incs): for instructions whose
        producers are >= 2 back in program order, the intervening
        instruction's execution covers the write-ack window."""
        if self.sem is not None:
            if wait:
                inst._wait_ge(self.sem, self.k)
            inst.then_inc(self.sem)
        self.k += 1
        return inst


def _emit_steps(eng, ch, AO, EK, iota, scr, acc, key, t0, t1, scol,
                relax=False):
    """Emit LZW steps t = t0..t1-1. scol(t) returns the AP pair
    (c_t column, (c_{t+1}+1)/16 column). relax=True drops the chain wait
    on the key update (its producer, the scan, is 2 instructions back)."""
    for t in range(t0, t1):
        c_col, cn_col = scol(t)
        if t >= 2:
            ch.add(eng.scalar_tensor_tensor(
                scr[:, 0:t], EK[:, 0:t], key[:], iota[:, 0:t],
                AO.is_equal, AO.mult, accum_out=acc[:]))
        ch.add(eng.scalar_tensor_tensor(
            EK[:, t:t + 1], acc[:], 0.0, key[:], AO.is_equal, AO.mult))
        ch.add(eng.scalar_tensor_tensor(
            key[:], acc[:], c_col, cn_col, AO.max, AO.add),
            wait=not (relax and t >= 2))


# instruction counts (for the chain-length assert / sync wait target)
_N_W0 = 5 + (3 * (T - 1) - 1) + 1 + 2          # header+steps+keysave+finals
_N_W1 = 1 + 3 * (T - PRE) + 1                  # key1 init + steps + final
_PER_REP = _N_W0 + _N_W1


def _build_program(use_chain=True, reps=1, relax=True):
    import concourse.bass as bass
    import concourse.mybir as mybir

    key_ = ("nc3", use_chain, reps, relax)
    if key_ in _nc_cache:
        return _nc_cache[key_]

    dt = mybir.dt.float32
    AO = mybir.AluOpType
    nc = bass.Bass()

    sym0_d = nc.declare_dram_parameter("syms0", [128, 2 * T], dt,
                                       isOutput=False)
    sym1_d = nc.declare_dram_parameter("syms1", [128, 2 * (T - PRE) + 2], dt,
                                       isOutput=False)
    out_d = nc.declare_dram_parameter("counts", [128, 3], dt, isOutput=True)

    sym0 = nc.alloc_sbuf_tensor("sym0", [128, 2 * T], dt).ap()
    sym1 = nc.alloc_sbuf_tensor("sym1", [128, 2 * (T - PRE) + 2], dt).ap()
    EK = nc.alloc_sbuf_tensor("EK", [128, T], dt).ap()
    scr = nc.alloc_sbuf_tensor("scr", [128, T], dt).ap()
    iota = nc.alloc_sbuf_tensor("iota", [128, T], dt).ap()
    acc = nc.alloc_sbuf_tensor("acc", [128, 1], dt).ap()
    key = nc.alloc_sbuf_tensor("key", [128, 1], dt).ap()
    key1 = nc.alloc_sbuf_tensor("key1", [128, 1], dt).ap()
    outt = nc.alloc_sbuf_tensor("outt", [128, 3], dt).ap()

    dma_sem = nc.alloc_semaphore("dma_sem")
    chain_sem = nc.alloc_semaphore("chain_sem")
    done_sem = nc.alloc_semaphore("done_sem")

    with nc.Block() as block:

        @block.sync
        def _(sync):
            sync.dma_start(sym0[:], sym0_d[:]).then_inc(dma_sem, 16)
            sync.dma_start(sym1[:], sym1_d[:]).then_inc(dma_sem, 16)
            if use_chain:
                sync.wait_ge(chain_sem, _PER_REP * reps)
            else:
                sync.wait_ge(done_sem, reps)
            sync.dma_start(out_d[:], outt[:]).then_inc(dma_sem, 16)

        @block.vector
        def _(vector):
            vector.wait_ge(dma_sem, 32)
            ch = _Chain(chain_sem if use_chain else None)

            def scol0(t):
                return (sym0[:, 2 * t:2 * t + 1],
                        sym0[:, 2 * t + 1:2 * t + 2])

            def scol1(t):
                u = t - PRE
                return (sym1[:, 2 * u + 1:2 * u + 2],
                        sym1[:, 2 * u + 2:2 * u + 3])

            for _ in range(reps):
                # ---- wave 0: full concat runs (string || pmap_{k0}) ----
                # iota = [8, 9, ..., T+7]: scr <- 1.0, then running add.
                ch.add(vector.memset(scr[:], 1.0))
                ch.add(vector.tensor_tensor_scan(
                    iota[:], scr[:], scr[:], 7.0, AO.add, AO.max))
                ch.add(vector.memset(acc[:], 0.0))
                ch.add(vector.memset(EK[:, 0:1], 0.0))
                ch.add(vector.scalar_tensor_tensor(
                    key[:], acc[:], sym0[:, 0:1], sym0[:, 1:2],
                    AO.max, AO.add))
                _emit_steps(vector, ch, AO, EK, iota, scr, acc, key,
                            1, PRE, scol0, relax=relax)
                # save key_{PRE} = cur_{PRE-1} + (c_PRE+1)/16 for wave 1
                ch.add(vector.tensor_copy(key1[:], key[:]))
                _emit_steps(vector, ch, AO, EK, iota, scr, acc, key,
                            PRE, T, scol0, relax=relax)
                # counts: nnz via (EK >= 0.03) min IOTA (IOTA >= 8 so
                # min(1,i)=1, min(0,i)=0), summed by accum_out.
                ch.add(vector.scalar_tensor_tensor(
                    scr[:, 0:PRE], EK[:, 0:PRE], 0.03, iota[:, 0:PRE],
                    AO.is_ge, AO.min, accum_out=outt[:, 0:1]))
                ch.add(vector.scalar_tensor_tensor(
                    scr[:], EK[:], 0.03, iota[:],
                    AO.is_ge, AO.min, accum_out=outt[:, 1:2]))

                # ---- wave 1: suffix-only runs (pmap_{k1}) reusing the
                # prefix trie state in EK[0:PRE] ----
                # key1 += delta, delta = (p'_0 - c_PRE)/16 (host-prestaged)
                ch.add(vector.scalar_tensor_tensor(
                    key1[:], key1[:], 1.0, sym1[:, 0:1], AO.mult, AO.add))
                _emit_steps(vector, ch, AO, EK, iota, scr, acc, key1,
                            PRE, T, scol1, relax=relax)
                last = ch.add(vector.scalar_tensor_tensor(
                    scr[:], EK[:], 0.03, iota[:],
                    AO.is_ge, AO.min, accum_out=outt[:, 2:3]))
                if not use_chain:
                    last.then_inc(done_sem)
            assert not use_chain or ch.k == _PER_REP * reps, (
                ch.k, _PER_REP * reps)

    _nc_cache[key_] = nc
    return nc


def _prestage_full(syms):
    """[n, T] symbols -> [n, 2T]: col 2t = c_t, col 2t+1 = (c_{t+1}+1)/16."""
    syms = np.asarray(syms, np.float32)
    n, T_ = syms.shape
    out = np.zeros((n, 2 * T_), np.float32)
    out[:, ::2] = syms
    out[:, 1:2 * T_ - 2:2] = (syms[:, 1:] + 1.0) / 16.0
    return out


def _prestage_suffix(syms, c_pre):
    """[n, S] suffix symbols + c_PRE column -> [n, 2S+2]:
    col 0 = (p'_0 - c_PRE)/16, col 1+2u = p'_u, col 2+2u = (p'_{u+1}+1)/16
    (0 for the last)."""
    syms = np.asarray(syms, np.float32)
    n, S = syms.shape
    out = np.zeros((n, 2 * S + 2), np.float32)
    out[:, 0] = (syms[:, 0] - np.asarray(c_pre, np.float32)) / 16.0
    out[:, 1::2][:, :S] = syms
    out[:, 2:2 * S - 1:2] = (syms[:, 1:] + 1.0) / 16.0
    return out


def _quantize(x, curve, levels):
    """x [B,C,H,W] -> strings [B,C,N] int32 (nearest level, first-min)."""
    out = np.asarray(x, np.float32).reshape(B, C, -1)[:, :, np.asarray(curve)]
    lv = np.asarray(levels, np.float32)
    return np.argmin(
        np.abs(out[:, :, None, :] - lv[:, None].reshape(1, C, L, 1)), axis=2
    ).astype(np.int32)


def _lane_symbols(strings, pmaps):
    """Per-core symbol matrices.

    Core n, lanes 0..95: bc = lane//8 (b = 4n + bc//3, c = bc%3),
    k0 = lane%8 (wave 0 concat), k1 = 8 + lane%8 (wave 1 suffix).
    Lanes 96..101 (wave 0): pmap-only runs cp = 6n..6n+5, zero-padded.
    Returns (syms0, syms1) lists of [128, *] f32 arrays."""
    pm = np.asarray(pmaps, np.int64)
    syms0, syms1 = [], []
    for n in range(NCORES):
        w0 = np.zeros((128, T), np.int64)
        w1 = np.zeros((128, T - PRE), np.int64)
        for lane in range(96):
            bc, k = lane // 8, lane % 8
            b_loc, c = bc // 3, bc % 3
            s = strings[4 * n + b_loc, c]
            w0[lane] = np.concatenate([s, pm[c, k]])
            w1[lane] = pm[c, 8 + k]
        for jj in range(6):
            cp = 6 * n + jj
            w0[96 + jj, :M] = pm[cp // 16, cp % 16]
        syms0.append(_prestage_full(w0))
        syms1.append(_prestage_suffix(w1, w0[:, PRE]))
    return syms0, syms1


def _assemble(results):
    """results[n]['counts'] [128, 3] -> ncd [32, 48] f32.
    cols: 0 = nnz(EK[0:PRE]) (c_s-1 / c_p-1), 1 = nnz(EK) after wave 0
    (c_sp-1, k0), 2 = nnz(EK) after wave 1 (c_sp-1, k1)."""
    c_s = np.zeros((B, C), np.float32)
    c_p = np.zeros((C, P), np.float32)
    c_sp = np.zeros((B, C, P), np.float32)
    for n in range(NCORES):
        cnts = np.asarray(results[n]["counts"], np.float32) + 1.0
        for lane in range(96):
            bc, k = lane // 8, lane % 8
            b_loc, c = bc // 3, bc % 3
            c_sp[4 * n + b_loc, c, k] = cnts[lane, 1]
            c_sp[4 * n + b_loc, c, 8 + k] = cnts[lane, 2]
            if k == 0:
                c_s[4 * n + b_loc, c] = cnts[lane, 0]
        for jj in range(6):
            cp = 6 * n + jj
            c_p[cp // 16, cp % 16] = cnts[96 + jj, 0]
    ncd = (c_sp - np.minimum(c_s[:, :, None], c_p[None, :, :])) / np.maximum(
        c_s[:, :, None], c_p[None, :, :])
    return ncd.reshape(B, C * P).astype(np.float32)


def _run(in_maps, trace=False, **build_kwargs):
    from concourse.bass_utils import run_bass_kernel_spmd
    nc = _build_program(**build_kwargs)
    return run_bass_kernel_spmd(nc, in_maps, list(range(NCORES)), trace=trace)


def _in_maps(x, curve, levels, pmaps):
    strings = _quantize(x, curve, levels)
    syms0, syms1 = _lane_symbols(strings, pmaps)
    return [{"syms0": syms0[n], "syms1": syms1[n]} for n in range(NCORES)]


def kernel(x, curve, levels, pmaps, i=0, **_unused):
    del i
    in_maps = _in_maps(x, curve, levels, pmaps)
    res = _run(in_maps)
    return _assemble([res.results[n] for n in range(NCORES)])


def kernel_profiled(x, curve, levels, pmaps, i=0, **_unused):
    """Like kernel() but with NTFF tracing; returns (out, exec_time_ns).
    Falls back to (out, None) when the profiling hook is unavailable."""
    del i
    in_maps = _in_maps(x, curve, levels, pmaps)
    try:
        res = _run(in_maps, trace=True)
        return (_assemble([res.results[n] for n in range(NCORES)]),
                res.exec_time_ns)
    except Exception:
        res = _run(in_maps)
        return _assemble([res.results[n] for n in range(NCORES)]), None


# revision 15
# speedup vs baseline: 1.1589x; 1.1589x over previous
"""NCD-via-LZW kernel for Trainium2 (8 NeuronCores, Bass) — v3.

Problem: quantize x [32,3,32,32] to 8 levels along a space-filling curve =>
96 strings of length 1024; LZW-compress the 96 strings, the 48 pattern maps,
and the 1536 string||pmap concatenations; return the normalized compression
distance matrix [32, 48].

Mapping: LZW is sequential per sequence but there are 1680 independent
sequences. Each NeuronCore handles batches 4n..4n+3 (192 concat runs) plus 6
of the 48 pmap runs, one LZW sequence per SBUF partition, on the Vector
engine. Wave 0 runs the full 2047-step concat string||pmap_{k0}; wave 1
reuses the trie state wave 0 left in SBUF (slots [0,1024) are exactly the
state after the shared 1024-symbol string prefix) and runs only the 1024
suffix steps for pmap_{k1}, overwriting slots [1024,2048) just ahead of its
own match stream.

v3 vs v1: the per-entry node id is its insertion slot + 8 (static), so the
match scan multiplies by a precomputed IOTA array instead of a maintained
EN array — 3 instructions per step instead of 4; compressed sizes are
end-of-run nonzero counts of EK instead of EN snapshots.

Per-lane LZW state (all exact in fp32):
  key(cur, c) = cur + (c+1)/16  (cur = trie node id, c in [0,8))
  EK[t] = key inserted at step t on miss, 0.0 on hit (queries >= 1/16 > 0)
Step t (query in key, match result in acc):
  1. acc = sum_j (EK[0:t] == key) * IOTA[0:t]     (IOTA[j] = j+8)
  2. EK[t] = (acc == 0) * key
  3. key = max(acc, c_t) + (c_{t+1}+1)/16         (node ids >= 8 > c_t)
lzw_count = 1 + nnz(EK[0:L]).
"""

import numpy as np

B, C, H, W = 32, 3, 32, 32
L = 8
P = 16
M = 1024
N = H * W
T = 2048
PRE = 1024
NCORES = 8

_nc_cache = {}


class _Chain:
    """Same-engine serialization via an attached-wait semaphore chain
    (required for correctness: back-to-back RAW on SBUF races the write
    ack — verified empirically)."""

    def __init__(self, sem):
        self.sem = sem
        self.k = 0

    def add(self, inst, wait=True):
        """wait=False skips the wait (still incs): for instructions whose
        producers are >= 2 back in program order, the intervening
        instruction's execution covers the write-ack window."""
        if self.sem is not None:
            if wait:
                inst._wait_ge(self.sem, self.k)
            inst.then_inc(self.sem)
        self.k += 1
        return inst


def _emit_steps(eng, ch, AO, EK, iota, scr, acc, key, t0, t1, scol,
                relax=False):
    """Emit LZW steps t = t0..t1-1. scol(t) returns the AP pair
    (c_t column, (c_{t+1}+1)/16 column). relax=True drops the chain wait
    on the key update (its producer, the scan, is 2 instructions back)."""
    for t in range(t0, t1):
        c_col, cn_col = scol(t)
        if t >= 2:
            ch.add(eng.scalar_tensor_tensor(
                scr[:, 0:t], EK[:, 0:t], key[:], iota[:, 0:t],
                AO.is_equal, AO.mult, accum_out=acc[:]))
        ch.add(eng.scalar_tensor_tensor(
            EK[:, t:t + 1], acc[:], 0.0, key[:], AO.is_equal, AO.mult))
        ch.add(eng.scalar_tensor_tensor(
            key[:], acc[:], c_col, cn_col, AO.max, AO.add),
            wait=not (relax and t >= 2))


# instruction counts (for the chain-length assert / sync wait target)
_N_W0 = 5 + (3 * (T - 1) - 1) + 1 + 2          # header+steps+keysave+finals
_N_W1 = 1 + 3 * (T - PRE) + 1                  # key1 init + steps + final
_PER_REP = _N_W0 + _N_W1


def _build_program(use_chain=True, reps=1, relax=True):
    import concourse.bass as bass
    import concourse.mybir as mybir

    key_ = ("nc3", use_chain, reps, relax)
    if key_ in _nc_cache:
        return _nc_cache[key_]

    dt = mybir.dt.float32
    AO = mybir.AluOpType
    nc = bass.Bass()

    sym0_d = nc.declare_dram_parameter("syms0", [128, 2 * T], dt,
                                       isOutput=False)
    sym1_d = nc.declare_dram_parameter("syms1", [128, 2 * (T - PRE) + 2], dt,
                                       isOutput=False)
    out_d = nc.declare_dram_parameter("counts", [128, 3], dt, isOutput=True)

    sym0 = nc.alloc_sbuf_tensor("sym0", [128, 2 * T], dt).ap()
    sym1 = nc.alloc_sbuf_tensor("sym1", [128, 2 * (T - PRE) + 2], dt).ap()
    EK = nc.alloc_sbuf_tensor("EK", [128, T], dt).ap()
    scr = nc.alloc_sbuf_tensor("scr", [128, T], dt).ap()
    iota = nc.alloc_sbuf_tensor("iota", [128, T], dt).ap()
    acc = nc.alloc_sbuf_tensor("acc", [128, 1], dt).ap()
    key = nc.alloc_sbuf_tensor("key", [128, 1], dt).ap()
    key1 = nc.alloc_sbuf_tensor("key1", [128, 1], dt).ap()
    outt = nc.alloc_sbuf_tensor("outt", [128, 3], dt).ap()

    dma_sem = nc.alloc_semaphore("dma_sem")
    dmb_sem = nc.alloc_semaphore("dmb_sem")
    chain_sem = nc.alloc_semaphore("chain_sem")
    done_sem = nc.alloc_semaphore("done_sem")

    with nc.Block() as block:

        @block.sync
        def _(sync):
            sync.dma_start(sym0[:], sym0_d[:]).then_inc(dma_sem, 16)
            sync.dma_start(sym1[:], sym1_d[:]).then_inc(dmb_sem, 16)
            if use_chain:
                sync.wait_ge(chain_sem, _PER_REP * reps)
            else:
                sync.wait_ge(done_sem, reps)
            sync.dma_start(out_d[:], outt[:]).then_inc(dma_sem, 16)

        @block.vector
        def _(vector):
            # start wave 0 as soon as sym0 lands; sym1 (wave-1 only)
            # transfers behind wave-0 compute and is waited on just before
            # its first use.
            vector.wait_ge(dma_sem, 16)
            ch = _Chain(chain_sem if use_chain else None)

            def scol0(t):
                return (sym0[:, 2 * t:2 * t + 1],
                        sym0[:, 2 * t + 1:2 * t + 2])

            def scol1(t):
                u = t - PRE
                return (sym1[:, 2 * u + 1:2 * u + 2],
                        sym1[:, 2 * u + 2:2 * u + 3])

            for _ in range(reps):
                # ---- wave 0: full concat runs (string || pmap_{k0}) ----
                # iota = [8, 9, ..., T+7]: scr <- 1.0, then running add.
                ch.add(vector.memset(scr[:], 1.0))
                ch.add(vector.tensor_tensor_scan(
                    iota[:], scr[:], scr[:], 7.0, AO.add, AO.max))
                ch.add(vector.memset(acc[:], 0.0))
                ch.add(vector.memset(EK[:, 0:1], 0.0))
                ch.add(vector.scalar_tensor_tensor(
                    key[:], acc[:], sym0[:, 0:1], sym0[:, 1:2],
                    AO.max, AO.add))
                _emit_steps(vector, ch, AO, EK, iota, scr, acc, key,
                            1, PRE, scol0, relax=relax)
                # save key_{PRE} = cur_{PRE-1} + (c_PRE+1)/16 for wave 1
                ch.add(vector.tensor_copy(key1[:], key[:]))
                _emit_steps(vector, ch, AO, EK, iota, scr, acc, key,
                            PRE, T, scol0, relax=relax)
                # counts: nnz via (EK >= 0.03) min IOTA (IOTA >= 8 so
                # min(1,i)=1, min(0,i)=0), summed by accum_out.
                ch.add(vector.scalar_tensor_tensor(
                    scr[:, 0:PRE], EK[:, 0:PRE], 0.03, iota[:, 0:PRE],
                    AO.is_ge, AO.min, accum_out=outt[:, 0:1]))
                ch.add(vector.scalar_tensor_tensor(
                    scr[:], EK[:], 0.03, iota[:],
                    AO.is_ge, AO.min, accum_out=outt[:, 1:2]))

                # ---- wave 1: suffix-only runs (pmap_{k1}) reusing the
                # prefix trie state in EK[0:PRE] ----
                # sym1 landed long ago (transfer overlapped wave 0)
                vector.wait_ge(dmb_sem, 16)
                # key1 += delta, delta = (p'_0 - c_PRE)/16 (host-prestaged)
                ch.add(vector.scalar_tensor_tensor(
                    key1[:], key1[:], 1.0, sym1[:, 0:1], AO.mult, AO.add))
                _emit_steps(vector, ch, AO, EK, iota, scr, acc, key1,
                            PRE, T, scol1, relax=relax)
                last = ch.add(vector.scalar_tensor_tensor(
                    scr[:], EK[:], 0.03, iota[:],
                    AO.is_ge, AO.min, accum_out=outt[:, 2:3]))
                if not use_chain:
                    last.then_inc(done_sem)
            assert not use_chain or ch.k == _PER_REP * reps, (
                ch.k, _PER_REP * reps)

    _nc_cache[key_] = nc
    return nc


def _prestage_full(syms):
    """[n, T] symbols -> [n, 2T]: col 2t = c_t, col 2t+1 = (c_{t+1}+1)/16."""
    syms = np.asarray(syms, np.float32)
    n, T_ = syms.shape
    out = np.zeros((n, 2 * T_), np.float32)
    out[:, ::2] = syms
    out[:, 1:2 * T_ - 2:2] = (syms[:, 1:] + 1.0) / 16.0
    return out


def _prestage_suffix(syms, c_pre):
    """[n, S] suffix symbols + c_PRE column -> [n, 2S+2]:
    col 0 = (p'_0 - c_PRE)/16, col 1+2u = p'_u, col 2+2u = (p'_{u+1}+1)/16
    (0 for the last)."""
    syms = np.asarray(syms, np.float32)
    n, S = syms.shape
    out = np.zeros((n, 2 * S + 2), np.float32)
    out[:, 0] = (syms[:, 0] - np.asarray(c_pre, np.float32)) / 16.0
    out[:, 1::2][:, :S] = syms
    out[:, 2:2 * S - 1:2] = (syms[:, 1:] + 1.0) / 16.0
    return out


def _quantize(x, curve, levels):
    """x [B,C,H,W] -> strings [B,C,N] int32 (nearest level, first-min)."""
    out = np.asarray(x, np.float32).reshape(B, C, -1)[:, :, np.asarray(curve)]
    lv = np.asarray(levels, np.float32)
    return np.argmin(
        np.abs(out[:, :, None, :] - lv[:, None].reshape(1, C, L, 1)), axis=2
    ).astype(np.int32)


def _lane_symbols(strings, pmaps):
    """Per-core symbol matrices.

    Core n, lanes 0..95: bc = lane//8 (b = 4n + bc//3, c = bc%3),
    k0 = lane%8 (wave 0 concat), k1 = 8 + lane%8 (wave 1 suffix).
    Lanes 96..101 (wave 0): pmap-only runs cp = 6n..6n+5, zero-padded.
    Returns (syms0, syms1) lists of [128, *] f32 arrays."""
    pm = np.asarray(pmaps, np.int64)
    syms0, syms1 = [], []
    for n in range(NCORES):
        w0 = np.zeros((128, T), np.int64)
        w1 = np.zeros((128, T - PRE), np.int64)
        for lane in range(96):
            bc, k = lane // 8, lane % 8
            b_loc, c = bc // 3, bc % 3
            s = strings[4 * n + b_loc, c]
            w0[lane] = np.concatenate([s, pm[c, k]])
            w1[lane] = pm[c, 8 + k]
        for jj in range(6):
            cp = 6 * n + jj
            w0[96 + jj, :M] = pm[cp // 16, cp % 16]
        syms0.append(_prestage_full(w0))
        syms1.append(_prestage_suffix(w1, w0[:, PRE]))
    return syms0, syms1


def _assemble(results):
    """results[n]['counts'] [128, 3] -> ncd [32, 48] f32.
    cols: 0 = nnz(EK[0:PRE]) (c_s-1 / c_p-1), 1 = nnz(EK) after wave 0
    (c_sp-1, k0), 2 = nnz(EK) after wave 1 (c_sp-1, k1)."""
    c_s = np.zeros((B, C), np.float32)
    c_p = np.zeros((C, P), np.float32)
    c_sp = np.zeros((B, C, P), np.float32)
    for n in range(NCORES):
        cnts = np.asarray(results[n]["counts"], np.float32) + 1.0
        for lane in range(96):
            bc, k = lane // 8, lane % 8
            b_loc, c = bc // 3, bc % 3
            c_sp[4 * n + b_loc, c, k] = cnts[lane, 1]
            c_sp[4 * n + b_loc, c, 8 + k] = cnts[lane, 2]
            if k == 0:
                c_s[4 * n + b_loc, c] = cnts[lane, 0]
        for jj in range(6):
            cp = 6 * n + jj
            c_p[cp // 16, cp % 16] = cnts[96 + jj, 0]
    ncd = (c_sp - np.minimum(c_s[:, :, None], c_p[None, :, :])) / np.maximum(
        c_s[:, :, None], c_p[None, :, :])
    return ncd.reshape(B, C * P).astype(np.float32)


def _run(in_maps, trace=False, **build_kwargs):
    from concourse.bass_utils import run_bass_kernel_spmd
    nc = _build_program(**build_kwargs)
    return run_bass_kernel_spmd(nc, in_maps, list(range(NCORES)), trace=trace)


def _in_maps(x, curve, levels, pmaps):
    strings = _quantize(x, curve, levels)
    syms0, syms1 = _lane_symbols(strings, pmaps)
    return [{"syms0": syms0[n], "syms1": syms1[n]} for n in range(NCORES)]


def kernel(x, curve, levels, pmaps, i=0, **_unused):
    del i
    in_maps = _in_maps(x, curve, levels, pmaps)
    res = _run(in_maps)
    return _assemble([res.results[n] for n in range(NCORES)])


def kernel_profiled(x, curve, levels, pmaps, i=0, **_unused):
    """Like kernel() but with NTFF tracing; returns (out, exec_time_ns).
    Falls back to (out, None) when the profiling hook is unavailable."""
    del i
    in_maps = _in_maps(x, curve, levels, pmaps)
    try:
        res = _run(in_maps, trace=True)
        return (_assemble([res.results[n] for n in range(NCORES)]),
                res.exec_time_ns)
    except Exception:
        res = _run(in_maps)
        return _assemble([res.results[n] for n in range(NCORES)]), None
